# revision 59
# baseline (speedup 1.0000x reference)
"""Additive (Bahdanau) attention weights on 8 TRN2 NeuronCores.

reference:
  qp = q @ W1.T ; kp = k @ W2.T + b_concat   (W1 = W_concat[:, :64], W2 = W_concat[:, 64:])
  logits[q,k] = sum_e w_logit[e] * tanh(qp[q,e] + kp[k,e]) + b_logit
  out = softmax(mask(logits), axis=k)        (b_logit drops: softmax shift-invariant)

Sharding: pure data-parallel, one (b, h) head per core (B*H = 8 = n_cores).
values is unused by the reference output.

Algorithm — order-1 Taylor in qp (|qp| < ~1, std 0.19), then use softmax's
invariance to per-q constants to drop the Sum_e wl*qp term:
  tanh(qp + kp) ~= t + (1 - t^2) qp,   t = tanh(kp)
  logits[q,k] ==softmax== sum_e wl[e] t[e,k] - sum_e (qp*wl)[q,e] t^2[e,k]
so the per-block matmul's moving operand is just [-t ; t^2]: tanh writes it
directly (no post-processing), one in-place DVE square per k-half, and the
wl factors fold into the stationary [(-wl)-broadcast ; qp*(-wl)].

Schedule (v3) highlights (11335 -> 10250 ns in the TimelineSim cost model):
- 3 input DMAs (k-side, q-side, mask[0:2]) hoisted to the very top of the SP
  prologue — above even the RegisterMoves, which only zero bounds-check
  registers that static-AP DMAs never read. First payload byte lands ~1.3us
  after t=0; mask[2:4] is issued first thing in the body and arrives in time
  for blocks 2,3.
- k-side data is fp8: kT and [W2T|W2T] ride as e4m3 bytes bitcast inside the
  bf16 payload, laid out for DoubleRow (d-dim = 32 partitions x 2 k-tiles),
  so the two kp projection matmuls run at 0.5 cycles/row (107ns at the
  mid p-state) — numpy cross-check puts the end-to-end cost of fp8 k/W2 at
  +0.3e-3 rel err. The DoubleRow identity for the mask matmuls rides the
  same DMA; masks are fp8 (0 / -40 exact) at half the bytes.
- Mask folded into the logits PSUM via identity matmuls on the idle PE
  (DoubleRow fp8: 107ns each); blocks 0,1 mask-first (mask half 0 lands
  early, fills idle PE slots), blocks 2,3 coeff-first so the PSUM-group
  start/stop order pins each late mask matmul behind its coeff matmul and
  it cannot head-of-line-block the PE.
- PSUM rotation bufs=3: block k's bank-reuse WAR naturally serializes late
  matmuls behind the exp that frees the bank (no scheduler hints needed).
- Row-sums: blocks 0-2 via a DVE tensor_scalar(*1+0) pass with accum_out
  (4x-mode, 194ns, hides under the 612ns ACT exp), block 3 via the ACT
  accumulator (shortest tail before the last reciprocal).
- exp0's critical path: c0's stationary (PP01 cols 0:128) is copied on ACT
  right after tanh2 (the ACT ack, not AA, binds c0); the qp projection is
  split so those 128 columns exist first; the remaining PP01 columns are
  scaled on DVE after the squares, in time for c1..c3.
- Outputs: blocks 0+1 leave as one pair-DMA at ot1 (their ready-times are
  one HWDGE slot apart anyway), blocks 2,3 as singles; the last DMA is
  small so the final drain waits only 364ns of transfer + fixed latencies.
Measured on-device rel err ~3.2e-3 (gate 2e-2).
"""

import numpy as np
import ml_dtypes

import concourse.bass as bass
import concourse.mybir as mybir
from concourse.tile import TileContext
from concourse.bass_utils import run_bass_kernel_spmd
from concourse.masks import make_identity

# ---------------------------------------------------------------------------
# Workaround: this walrus build allows only ONE sync-wait per instruction, but
# Tile's semaphore pass sometimes emits 2-3 on one instruction. Post-process
# the module: hoist extra waits onto standalone Drain instructions spliced in
# directly before the violating instruction (same engine, so the per-engine
# program order enforces the waits before it executes).


def _split_multiwaits(nc):
    for fn in nc.m.functions:
        for blk in fn.blocks:
            insts = list(blk.instructions)
            newlist = []
            changed = False
            for inst in insts:
                si = inst.sync_info
                if si is not None and si.on_wait and len(si.on_wait) > 1:
                    waits = list(si.on_wait)
                    for w in waits[:-1]:
                        d = mybir.InstDrain(
                            name=nc.get_next_instruction_name(),
                            ins=[],
                            outs=[],
                            bass_is_fusable=False,
                        )
                        d.engine = inst.engine
                        d.sync_info = mybir.SyncInfo(on_wait=[w], on_update=[])
                        nc.register_instruction(d)
                        newlist.append(d)
                    inst.sync_info = mybir.SyncInfo(
                        on_wait=[waits[-1]], on_update=list(si.on_update or [])
                    )
                    changed = True
                newlist.append(inst)
            if changed:
                blk.instructions = newlist
# ---------------------------------------------------------------------------
# The Tile prologue ends with an all-engine barrier (~1us in) before the body
# issues its first DMA. The input DMAs only use SP's own HWDGE ring (set up by
# SP's RegisterMoves, which precede them in SP program order) and their
# completion semaphores are runtime-initialized and untouched by the prologue,
# so they can issue BEFORE the barrier: hoist them from the body block into
# the prologue block, right before SP's barrier Drain.


def _hoist_input_dmas(nc, n=3):
    fn = nc.m.functions[0]
    pro, body = fn.blocks[0], fn.blocks[1]
    moved = []
    kept = []
    for inst in body.instructions:
        if (
            len(moved) < n
            and type(inst).__name__ == "InstDMACopy"
            and inst.engine == mybir.EngineType.SP
            and not (inst.sync_info and inst.sync_info.on_wait)
        ):
            moved.append(inst)
        else:
            kept.append(inst)
    if not moved:
        return
    body.instructions = kept
    # insert at the very top: SP's prologue RegisterMoves only zero scratch /
    # bounds-check registers that static-AP DMAs never read
    pro.instructions = moved + list(pro.instructions)
# ---------------------------------------------------------------------------
# The Tile epilogue runs ~3 all-engine barrier rounds (~430ns serial) after
# the SP Drains that wait out the DMA-completion semaphores. For a
# single-shot kernel only the SP Drains are load-bearing: SP halts last,
# after every output DMA's semaphore; other engines may halt early. Strip
# the barrier rounds (everything in the epilogue block that isn't an SP
# Drain waiting a data/DMA semaphore).


def _strip_epilogue_barriers(nc):
    epi = nc.m.functions[0].blocks[-1]
    keep = []
    for inst in epi.instructions:
        si = inst.sync_info
        is_data_drain = (
            inst.engine == mybir.EngineType.SP
            and type(inst).__name__ == "InstDrain"
            and si is not None
            and si.on_wait
            and all(w.id not in (151, 152) for w in si.on_wait)
            and not si.on_update
        )
        if is_data_drain:
            keep.append(inst)
    epi.instructions = keep
# ---------------------------------------------------------------------------

F32 = mybir.dt.float32
BF16 = mybir.dt.bfloat16
F8 = mybir.dt.float8e4  # e4m3
AF = mybir.ActivationFunctionType
ALU = mybir.AluOpType
PerfMode = mybir.MatmulPerfMode if hasattr(mybir, "MatmulPerfMode") else None

B, H, LQ, LKV, D = 2, 4, 512, 512, 64
NCORES = 8
NBLK = LQ // 128


def build_program(n_reps=1):
    nc = bass.Bass()
    # k-side: [ kT(512) | W2T | W2T (128) | wl | -wl | pad | DoubleRow-identity
    # as raw fp8 bytes packed into 128 bf16 cols ]  -> qkw[64:128, :]
    kw_d = nc.declare_dram_parameter("kw", [64, 771], BF16, isOutput=False)
    # q-side: [ qT(512) | W1T (64) | pad | wl | -wl | pad ] -> qkw[0:64, :]
    qw_d = nc.declare_dram_parameter("qw", [64, 643], BF16, isOutput=False)
    # additive mask, fp8 e4m3, 0 keep / -40 drop, DoubleRow layout:
    # [p(64), blk, half, k] = mask row (64*half + p) of block blk
    m01_d = nc.declare_dram_parameter("m01", [64, 2, 2, 512], F8, isOutput=False)
    m23_d = nc.declare_dram_parameter("m23", [64, 2, 2, 512], F8, isOutput=False)
    out_d = nc.declare_dram_parameter("out", [LQ, LKV], BF16, isOutput=True)

    with TileContext(nc) as tc:
        with (
            tc.tile_pool(name="const", bufs=1) as cpool,
            tc.tile_pool(name="mwork", bufs=6) as m_pool,
            tc.tile_pool(name="small", bufs=8) as s_pool,
            tc.tile_pool(name="lpsum", bufs=3, space="PSUM") as lps_pool,
            tc.tile_pool(name="prep_psum", bufs=1, space="PSUM") as pp,
        ):
            # ------------- input DMAs (hoisted pre-barrier: first 3 on SP) --
            qkw = cpool.tile([128, 771], BF16)
            # mask lives on partitions 64:128 (same base as the identity,
            # which rides in kw's cols 643:771 as raw fp8 bytes)
            mneg = cpool.tile([128, 4, 2, 512], F8)
            nc.sync.dma_start(out=qkw[64:128, :], in_=kw_d[:])
            nc.sync.dma_start(out=qkw[0:64, 0:643], in_=qw_d[:])
            nc.sync.dma_start(out=mneg[64:128, 0:2, :, :], in_=m01_d[:])
            # body-issued (4th): lands in time for blocks 2,3
            nc.sync.dma_start(out=mneg[64:128, 2:4, :, :], in_=m23_d[:])
            identf8 = qkw[64:128, 643:771].bitcast(F8).rearrange(
                "p (h q) -> p h q", h=2
            )   # [64, 2, 128] DoubleRow identity

            qkt = qkw[:, 0:512]
            S2 = qkw[0:64, 512:576]     # [64,64]  = W1T,        c=64 over qT
            # k-side data rides as fp8 (DoubleRow: d-dim = 32 partitions x 2
            # k-tiles) packed into kw's bf16 payload on partitions 64:96
            kTf8 = qkw[64:96, 0:512].bitcast(F8).rearrange(
                "p (h k) -> p h k", h=2)        # [32, 2, 512]
            S1f8 = qkw[64:96, 512:640].bitcast(F8).rearrange(
                "p (h m) -> p h m", h=2)        # [32, 2, 128] = [W2T|W2T]

            # ------------- constants --------------------------------------
            # scalars [wl | -wl] as f32, all 128 partitions. gpsimd
            # (software engine) can copy across partitions and cast.
            # (b_concat is folded into keys on the host: k' = k + W2^-T bc.)
            wb = s_pool.tile([128, 3], F32, tag="wb")
            nc.gpsimd.tensor_copy(wb[64:128, :], qkw[64:128, 640:643])
            nc.gpsimd.tensor_copy(wb[0:64, :], qkw[64:128, 640:643])
            wn2 = wb[:, 1:2]

            # Softmax is shift-invariant in per-q constants, so
            #   logits ==_softmax  sum_e wl[e] t[e,k] - sum_e (qp.wl)[q,e] t^2[e,k]
            # Moving operand AA01 = [-t ; t^2] (tanh writes it directly, one
            # in-place square per half); stationary PP01 = [-wl bcast ; qp*wl].
            PP01 = cpool.tile([128, 512], BF16)
            nc.vector.memset(PP01[0:64, :], 1.0)
            nc.vector.tensor_scalar_mul(PP01[0:64, :], PP01[0:64, :],
                                        wn2[0:64, :])

            # ------------- projections ------------------------------------
            # p2a = [kpT ; kpT] in two k-half tiles, p2b = [* ; qpT]
            p2a1 = pp.tile([128, 512], F32, name="p2a1")
            p2a2 = pp.tile([128, 512], F32, name="p2a2")
            p2b = pp.tile([128, 512], F32, name="p2b")
            nc.tensor.matmul(p2a1[:, 0:256], S1f8, kTf8[:, :, 0:256],
                             start=True, stop=True,
                             perf_mode=mybir.MatmulPerfMode.DoubleRow)
            nc.tensor.matmul(p2a2[:, 0:256], S1f8, kTf8[:, :, 256:512],
                             start=True, stop=True,
                             perf_mode=mybir.MatmulPerfMode.DoubleRow)
            # qp proj split: block 0's 128 q-cols first, so the PP01 scaled
            # copy (c0's stationary) can start as early as possible
            nc.tensor.matmul(p2b[64:128, 0:128], S2, qkt[0:64, 0:128], start=True, stop=True)
            nc.tensor.matmul(p2b[64:128, 128:512], S2, qkt[0:64, 128:512], start=True, stop=True)

            # ------------- coefficients (k-halved pipeline) ---------------
            # tanh writes AA01 = [-t ; -t] directly; an in-place square on
            # rows 64:128 turns the lower copy into t^2. Rows 0:64 keep -t
            # (PP01's -wl broadcast restores the + sign in the matmul).
            AA01 = cpool.tile([128, 512], BF16)
            for hi, ((h0, h1), p2ah) in enumerate(
                (((0, 256), p2a1), ((256, 512), p2a2))
            ):
                nc.scalar.activation(AA01[:, h0:h1], p2ah[:, 0:256], AF.Tanh,
                                     scale=-1.0)
                nc.vector.tensor_mul(
                    AA01[64:128, h0:h1], AA01[64:128, h0:h1],
                    AA01[64:128, h0:h1]
                )

            # PP01 rows 64:128 = qp * (-wl): block 0's stationary columns via
            # a scaled copy on ACT right after tanh2 (c0 is the critical
            # consumer), the rest on DVE right after the squares.
            nc.scalar.activation(PP01[64:128, 0:128], p2b[64:128, 0:128],
                                 AF.Copy, scale=wn2[64:128, :])
            nc.vector.tensor_scalar_mul(PP01[64:128, 128:256],
                                        p2b[64:128, 128:256],
                                        wn2[64:128, :])
            nc.vector.tensor_scalar_mul(PP01[64:128, 256:512],
                                        p2b[64:128, 256:512],
                                        wn2[64:128, :])

            # ------------- blocks: matmuls + softmax ----------------------
            for _rep in range(n_reps):
                banks = [
                    lps_pool.tile([128, 512], F32, tag="lps", name=f"lps{blk}")
                    for blk in range(NBLK)
                ]

                def mask_mm(blk, start, stop):
                    nc.tensor.matmul(
                        banks[blk][:], identf8, mneg[64:128, blk, :, :],
                        start=start, stop=stop,
                        perf_mode=mybir.MatmulPerfMode.DoubleRow,
                    )

                def coeff_mm(blk, start, stop, k0=0, k1=512):
                    nc.tensor.matmul(
                        banks[blk][:, k0:k1],
                        PP01[:, blk * 128 : blk * 128 + 128],
                        AA01[:, k0:k1], start=start, stop=stop,
                    )

                # blocks 0,1: mask first (half 0 lands early); 2,3: coeff
                # first, with the late-landing mask matmuls scheduled last
                # (tile_wait_until) so they can't head-of-line-block the PE
                # in front of c0/c1.
                # blocks 0,1: mask first (half 0 lands early; the mask
                # matmuls fill otherwise-idle PE slots before c0's data is
                # ready). Blocks 2,3: coeff first — the PSUM group start/stop
                # order then forces each late-landing mask matmul behind its
                # coeff matmul, so it cannot head-of-line-block the PE.
                mask_mm(0, True, False)
                coeff_mm(0, False, True)
                mask_mm(1, True, False)
                coeff_mm(1, False, True)
                coeff_mm(2, True, False)
                mask_mm(2, False, True)
                coeff_mm(3, True, False)
                mask_mm(3, False, True)

                sscr = m_pool.tile([128, 512], BF16, tag="sscr")
                # blocks 0+1 leave as one pair-DMA at ot1 (saves a shared
                # HWDGE slot; their transfer has plenty of slack), blocks 2,3
                # as singles so the last DMA is small and data-bound.
                opair = m_pool.tile([128, 2, 512], BF16, tag="opair")
                for blk in range(NBLK):
                    lb = banks[blk]
                    # |logits| <= ||w_logit||_1 ~ 1.3 -> exp cannot overflow;
                    # masked entries are exp(l - 40) ~ 0
                    et = m_pool.tile([128, 512], BF16, tag="et")
                    ssum = s_pool.tile([128, 1], F32, tag="ssum")
                    if blk == NBLK - 1:
                        # last block: ACT accumulator -> shortest tail
                        nc.scalar.activation(et[:], lb[:], AF.Exp,
                                             accum_out=ssum[:, 0:1])
                    else:
                        nc.scalar.activation(et[:], lb[:], AF.Exp)
                        # row-sum on DVE: dummy *1+0 pass with accumulator
                        nc.vector.tensor_scalar(
                            out=sscr[:], in0=et[:],
                            scalar1=1.0, scalar2=0.0,
                            op0=ALU.mult, op1=ALU.add,
                            accum_out=ssum[:, 0:1],
                        )
                    rs = s_pool.tile([128, 1], F32, tag="rs")
                    nc.vector.reciprocal(rs[:], ssum[:])
                    if blk < 2:
                        ot = opair[:, blk, :]
                        nc.vector.tensor_scalar_mul(ot, et[:], rs[:, 0:1])
                        if blk == 1:
                            nc.sync.dma_start(
                                out=out_d[0:256, :]
                                .rearrange("(t p) k -> p t k", p=128),
                                in_=opair[:],
                            )
                    else:
                        ot = m_pool.tile([128, 512], BF16, tag="ot")
                        nc.vector.tensor_scalar_mul(ot[:], et[:], rs[:, 0:1])
                        nc.sync.dma_start(
                            out=out_d[blk * 128 : blk * 128 + 128, :],
                            in_=ot[:],
                        )
    _hoist_input_dmas(nc, n=3)
    _strip_epilogue_barriers(nc)
    _split_multiwaits(nc)
    return nc


_NC_CACHE = None


def _get_program():
    global _NC_CACHE
    if _NC_CACHE is None:
        _NC_CACHE = build_program()
    return _NC_CACHE


def kernel(queries, keys, values=None, mask=None, W_concat=None, b_concat=None,
           w_logit=None, b_logit=None, **_unused):
    queries = np.asarray(queries, dtype=np.float32)
    keys = np.asarray(keys, dtype=np.float32)
    mneg = (np.asarray(mask).astype(np.float32) - 1.0) * 40.0  # 0 keep / -40 drop
    wc = np.asarray(W_concat, dtype=np.float32)
    w1t = np.ascontiguousarray(wc[:, :D].T)   # [d, e] = W1[e, d]
    w2t = np.ascontiguousarray(wc[:, D:].T)
    wl = np.asarray(w_logit, dtype=np.float32).reshape(D, 1)
    bc = np.asarray(b_concat, dtype=np.float32).reshape(D)
    # b_logit shifts all logits equally -> cancels in softmax. values unused.
    # Fold b_concat into keys (parameter-only solve; k' = k + W2^-T bc gives
    # W2^T k' = W2^T k + bc exactly). bc is zeros here, so this is inert.
    if np.any(bc != 0.0):
        keys = keys + np.linalg.solve(wc[:, D:], bc)[None, None, None, :]

    bf = ml_dtypes.bfloat16
    f8 = ml_dtypes.float8_e4m3
    nc = _get_program()
    # DoubleRow identity payload: ident[p, h, q] = (64*h + p == q),
    # packed as raw fp8 bytes into 128 bf16 columns of kw (bitcast on-chip)
    identp = np.zeros((64, 2, 128), np.float32)
    for hh in range(2):
        identp[np.arange(64), hh, 64 * hh + np.arange(64)] = 1.0
    identbits = (
        identp.astype(f8).view(np.uint8).reshape(64, 256)
        .view(np.uint16).view(bf)
    )  # [64, 128] bf16 carrying the fp8 bytes
    w2dup = np.concatenate([w2t, w2t], axis=1)  # [64(d), 128]
    s1bits = (
        w2dup.reshape(2, 32, 128).transpose(1, 0, 2).astype(f8)
        .view(np.uint8).reshape(32, 256).view(np.uint16).view(bf)
    )  # [32, 128] bf16 carrying [2,128] fp8 rows
    in_maps = []
    for c in range(NCORES):
        b, h = divmod(c, H)
        kw = np.zeros((64, 771), np.float32).astype(bf)
        # kT as fp8 DoubleRow payload: [p(32), tile(2), k] = kT[32*t+p, k]
        kt = keys[b, h].T  # [64(d), 512]
        kw[0:32, 0:512] = (
            kt.reshape(2, 32, 512).transpose(1, 0, 2).astype(f8)
            .view(np.uint8).reshape(32, 1024).view(np.uint16).view(bf)
        )
        kw[0:32, 512:640] = s1bits
        kw[:, 640:641] = wl.astype(bf)
        kw[:, 641:642] = (-wl).astype(bf)
        kw[:, 643:771] = identbits
        qw = np.zeros((64, 643), np.float32)
        qw[:, 0:512] = queries[b, h].T
        qw[:, 512:576] = w1t
        qw[:, 640:641] = wl
        qw[:, 641:642] = -wl
        # DoubleRow mask layout: [p(64), blk, half, k] = mask row
        # 64*half+p of block blk
        mcore = mneg[b].reshape(4, 2, 64, 512).transpose(2, 0, 1, 3)
        in_maps.append(
            {
                "kw": kw,
                "qw": qw.astype(bf),
                "m01": np.ascontiguousarray(mcore[:, 0:2, :, :]).astype(f8),
                "m23": np.ascontiguousarray(mcore[:, 2:4, :, :]).astype(f8),
            }
        )
    global _last_in_maps
    _last_in_maps = in_maps
    res = run_bass_kernel_spmd(nc, in_maps, list(range(NCORES)))
    out = np.stack(
        [np.asarray(res.results[c]["out"], dtype=np.float32) for c in range(NCORES)]
    )
    return out.reshape(B, H, LQ, LKV)


_last_in_maps = None
